# revision 12
# baseline (speedup 1.0000x reference)
"""Trainium2 Bass kernel for nn_Correlation: -mean(einsum('itj,itl->ijl', x, y)).

Math: mean over [B, C, C] of corr[b,j,l] = sum_t x[b,t,j] y[b,t,l] equals
  (1/(B*C^2)) * sum_{b,t} (sum_j x[b,t,j]) * (sum_l y[b,t,l])
so the kernel only needs per-row sums of x and y plus a dot product —
a pure memory-bound streaming reduction (no matmul).

Sharding: data-parallel over batch. 8 cores, 1 batch element each.
The host casts inputs to bf16 (tolerance is 2e-2; measured rel err
1.6e-3), halving HBM traffic to 4 MB per tensor per core. Each core
streams x[b], y[b] through SBUF; the two HWDGE rings together sustain
~425 GB/s. Compute is element-bound, not byte-bound (DVE tensor_reduce
~1.07 us and ACT activation ~1.41 us per 1024-elem row, dtype-blind),
so chunk sizes are shaped to keep both reducers fed: x reduces on DVE
(descending chunks, small tail); y reduces on ACT (leading small chunk
so the serial 1.41 us/row activation chain starts as soon as the first
rows land). Chunk completions pace at the slowest SDMA engine (#15,
~25% slower) — arrivals ~1.44 us/row-unit. Row sums land f32 in one
[128, 2, 16] tile; two single-wait stores (y via SWDGE early, x on the
last fresh HWDGE lane) hand them to the host, which un-permutes,
multiplies x/y row sums, sums, and scales.

Constraints honored (this walrus build allows ONE sync wait per
instruction — verified empirically, even for Drain):
- every chunk gets a dedicated SBUF slot (no WAR/WAW waits on loads);
- activation writes in place (a scratch tile's WAW reuse would add a
  second wait);
- 7 loads + 2 stores split so each DMA carries exactly one wait;
- TileContext's tail drain is split into one drain per proc lane
  (_patch_tail_drain).
"""

import numpy as np

B, T, C = 8, 2048, 1024
P = 128             # SBUF partitions
RPP = T // P        # rows per partition (16)
# rows/partition per chunk (each sums to RPP): descending sizes — large
# chunks sustain HBM bandwidth, small final chunks shorten the reduce
# tail after the stream ends. 4+3 = 7 loads leaves one HWDGE completion
# lane fresh for the x store.
# 7 loads total is a hard cap: every HWDGE DMA beyond 8 (7 loads + the x
# store) lands on a reused completion lane and picks up a second sync
# wait, which this build rejects ("Too many sync wait commands").
# x (DVE tensor_reduce) descends so the post-stream reduce tail is
# short; y (ACT activation chain, 1.41us/row serial) leads with a small
# chunk so ACT starts early and pipelines with the stream.
XCHUNKS = [8, 6, 2]
YCHUNKS = [2, 5, 5, 4]
N_CORES = 8

_CACHE = {}


def _patch_tail_drain(tile):
    """Split TileContext's kernel-tail drain into one drain per proc lane.

    The stock tail emits a single SP Drain waiting on every outstanding
    sem (DVE + ACT + each DMA completion lane); this walrus build caps
    sync waits per instruction below that, so codegen fails with "Too
    many sync wait commands". Waiting on the sems one drain at a time is
    equivalent (SP program order) and keeps every instruction at 1 wait.
    """
    import re
    import bass_rust
    from concourse.vector_clock import ScopedClock

    if getattr(tile.TileContext, "_tail_drain_split", False):
        return

    def _drain_and_barrier(self, tick_clock, wait_clock):
        ticks = [int(s) for s in re.findall(r"-?\d+",
                                            repr(tick_clock.global_clock))]
        # Transitive closure: the only sems NOT implied by others are the two
        # store completion lanes — store_y is the sole SWDGE DMA (DMASW0,
        # proc 11) and store_x the 8th HWDGE DMA (lane DMAHW7, proc 26).
        # store_x waited on DVE, store_y on ACT, and every reduce waited on
        # its load lane, so waiting on the store lanes covers everything.
        # VALID ONLY for exactly 7 HWDGE loads + 1 SWDGE store + 1 HWDGE
        # store (len(XCHUNKS)+len(YCHUNKS)==7): with 8+ HWDGE DMAs, DMAHW7
        # would be a load lane and this would skip a store wait. Fall back
        # to draining every lane otherwise.
        minimal = [11, 26]
        n_hwdge = len(XCHUNKS) + len(YCHUNKS) + 1  # loads + store_x
        if n_hwdge == 8 and all(
                0 <= i < len(ticks) and ticks[i] > 0 for i in minimal):
            lanes = minimal
        else:
            lanes = [i for i, t in reversed(list(enumerate(ticks))) if t > 0]
        for i in lanes:
            part = bass_rust.VectorClock(
                [ticks[i] if j == i else 0 for j in range(len(ticks))])
            d = self.nc.sync.drain()
            wait_clock.add_sem_waits(d.ins, ScopedClock({None: part}))
        self.nc.all_engine_barrier()
        assert self.sems is not None
        popped = self.nc._tile_sem_poison_stack.pop()
        assert popped is self._sem_poison
        # no second barrier: the NRT postamble's full sem sweep makes any
        # clear-vs-postamble write race benign (both write zero)
        self.nc.clear_and_free_semaphores(list(self.sems.allocated().values()))

    tile.TileContext._drain_and_barrier = _drain_and_barrier
    tile.TileContext._tail_drain_split = True


def _build_bass():
    import concourse.bass as bass
    import concourse.tile as tile
    from concourse import mybir

    _patch_tail_drain(tile)

    f32 = mybir.dt.float32
    bf16 = mybir.dt.bfloat16
    # Bass.__init__ unconditionally memsets a const pool and emits an
    # all-engine barrier (~0.7 us on the measured critical path). This
    # kernel never reads the const APs, so suppress both during init.
    _ob, _om = bass.Bass.all_engine_barrier, bass.BassSharedVectorInterface.memset
    bass.Bass.all_engine_barrier = lambda self, *a, **k: None
    bass.BassSharedVectorInterface.memset = lambda self, *a, **k: None
    try:
        nc = bass.Bass()
    finally:
        bass.Bass.all_engine_barrier = _ob
        bass.BassSharedVectorInterface.memset = _om
    x = nc.dram_tensor("x", [T, C], bf16, kind="ExternalInput")
    y = nc.dram_tensor("y", [T, C], bf16, kind="ExternalInput")
    out = nc.dram_tensor("out", [P, 2, RPP], f32, kind="ExternalOutput")

    with tile.TileContext(nc) as tc:
        with (
            # dedicated slot per chunk (unique tags, 1 buf each): load DMAs
            # never carry WAR/WAW waits
            tc.tile_pool(name="iox", bufs=1) as iox,
            tc.tile_pool(name="ioy", bufs=1) as ioy,
            tc.tile_pool(name="acc", bufs=1) as acc,
        ):
            sxy = acc.tile([P, 2, RPP], f32)  # [:,0,:] x sums, [:,1,:] y sums

            # all load triggers first: x on the SP ring, y on the ACT ring
            # (two rings stream faster than one; pre-issuing keeps the y
            # triggers ahead of the slow activations in ACT program order)
            xts, yts = [], []
            offx = offy = 0
            for i in range(max(len(XCHUNKS), len(YCHUNKS))):
                if i < len(YCHUNKS):
                    a = YCHUNKS[i]
                    yt = ioy.tile([P, a, C], bf16, tag=f"yt{offy}")
                    # y0 rides at the head of the SP ring: the ACT ring
                    # starts streaming ~3us late, so the first (tiny) y
                    # chunk would otherwise gate the whole serial ACT
                    # chain on that lag.
                    eng = nc.sync if i == 0 else nc.scalar
                    eng.dma_start(
                        out=yt[:],
                        in_=y[offy * P:(offy + a) * P, :]
                            .rearrange("(p a) c -> p a c", p=P))
                    yts.append((offy, a, yt))
                    offy += a
                if i < len(XCHUNKS):
                    a = XCHUNKS[i]
                    xt = iox.tile([P, a, C], bf16, tag=f"xt{offx}")
                    nc.sync.dma_start(
                        out=xt[:],
                        in_=x[offx * P:(offx + a) * P, :]
                            .rearrange("(p a) c -> p a c", p=P))
                    xts.append((offx, a, xt))
                    offx += a

            for off, a, xt in xts:
                nc.vector.tensor_reduce(
                    out=sxy[:, 0, off:off + a], in_=xt[:],
                    axis=mybir.AxisListType.X, op=mybir.AluOpType.add,
                )
            for off, a, yt in yts:
                for j in range(a):
                    nc.scalar.activation(
                        out=yt[:, j], in_=yt[:, j],
                        func=mybir.ActivationFunctionType.Copy,
                        accum_out=sxy[:, 1, off + j:off + j + 1],
                    )

            # each store carries ONE wait. x half (DVE finishes first) goes
            # via SWDGE so its ~1.8us completion latency hides under the ACT
            # tail; y half (last thing ready) takes the fresh HWDGE lane for
            # the quickest trigger-to-completion on the critical path.
            nc.gpsimd.dma_start(out=out[:, 0], in_=sxy[:, 0])
            nc.sync.dma_start(out=out[:, 1], in_=sxy[:, 1])
    return nc


def _run(x, y, trace=False):
    from concourse.bass_utils import run_bass_kernel_spmd

    if "nc" not in _CACHE:
        _CACHE["nc"] = _build_bass()
    nc = _CACHE["nc"]
    import ml_dtypes
    bf16 = ml_dtypes.bfloat16
    in_maps = [
        {"x": np.ascontiguousarray(x[i].astype(bf16)),
         "y": np.ascontiguousarray(y[i].astype(bf16))}
        for i in range(N_CORES)
    ]
    return run_bass_kernel_spmd(nc, in_maps, core_ids=list(range(N_CORES)),
                                trace=trace)


def _row_map(chunks):
    """row index for each (partition, column) of the on-chip sum tile:
    chunk at column offset `off` with `a` rows/partition holds row
    off*P + p*a + j in column off+j."""
    m = np.empty((P, RPP), np.int64)
    off = 0
    for a in chunks:
        for j in range(a):
            m[:, off + j] = off * P + np.arange(P) * a + j
        off += a
    return m


_XMAP = _row_map(XCHUNKS)
_YMAP = _row_map(YCHUNKS)


def kernel(**inputs) -> np.ndarray:
    x = np.asarray(inputs["x"], dtype=np.float32)
    y = np.asarray(inputs["y"], dtype=np.float32)
    res = _run(x, y, trace=False)
    s = 0.0
    for r in res.results:
        o = r["out"].astype(np.float64)
        sx = np.empty(T); sx[_XMAP.ravel()] = o[:, 0, :].ravel()
        sy = np.empty(T); sy[_YMAP.ravel()] = o[:, 1, :].ravel()
        s += (sx * sy).sum()
    return np.array(-s / (B * C * C), dtype=np.float32)

